# revision 62
# baseline (speedup 1.0000x reference)
"""Trainium2 Bass kernel for nn_Attention_14542759264705.

Dense transformer attention: QKV proj + interleaved RoPE + GQA causal
attention (32 q heads / 8 kv heads, hd=64) + output proj, fp32 in/out.

Sharding: tensor-parallel over kv-head groups across 8 cores. Core c owns
q heads 4c..4c+3 and kv head c; each core computes a partial output and
the host sums the 8 partials.

v3 (vs the v2 two-phase baseline), measured 350us vs 491us:
  - Single fused PE stream: projection j-chunks are interleaved with
    attention pairs (proj0, proj1, A00, proj2, A01, ... A13) so the PE
    never parks at phase/pair boundaries (v2 lost ~146us to HAM
    re-throttling during starvation windows; each idle gap >2us costs
    the gap plus a ~3.4us half-clock window after it).
  - Warm-up matmuls (no DMA deps) lift the HAM clock gate to 2.4GHz
    while the first x tiles are still in flight.
  - Dual-queue DMA: Sync carries x tiles + o writes + norm reshapes,
    Scalar carries the weight preload; x is stored j-major so each
    512-token chunk is a linear read (4x512KB chunks when prefetched).
  - Causal trim at 128-col granularity for scores, exp, and PV; the
    tri-mask multiply is a single [128,2,128] DVE op per diagonal tile.
  - Per-pi softmax-denominator chains ([1,1024] -> DMA-reshape [8,128]
    -> ACT Ln/Exp -> bf16 broadcast) deferred into the next kt loops;
    the final pair instead runs Ln/Exp directly on the denominator row
    and broadcasts via a K=1 PE outer product (DMA hops cost ~2us of
    completion-receipt latency each, which the tail cannot hide).
  - RoPE DVE work for chunk j is emitted MID-pair (between pi0 and pi1
    of the pair that follows proj(j)) so it never sits ahead of norm
    muls or mask muls on the in-order DVE queue.
  - pi1's first two score tiles are hoisted before pi0's PV flush to
    cover the trailing exps; wo steps drip-feed through the shared
    psum pool with drains emitted BEFORE each kt's score (their casts
    must not delay the mask mul that gates the PV).
  - A 6-step wo reserve from the next-to-last pair is released at
    high_priority before the last pair's norm chain, giving the PE
    work during the tail's serial norm latency.
"""
import numpy as np

B, S, D = 2, 2048, 2048
T = B * S
NH, NKV, HD = 32, 8, 64
NCORES = 8

_cache = {}


def _build():
    from collections import deque
    from itertools import cycle

    import concourse.bacc as bacc
    import concourse.mybir as mybir
    import concourse.tile as tile
    from concourse.masks import make_identity

    F32 = mybir.dt.float32
    BF16 = mybir.dt.bfloat16
    AF = mybir.ActivationFunctionType

    # Force Exp/Ln/Copy onto the single combined act table set so the
    # compiler never inserts per-call ACT_TABLE_LOADs between exps and
    # the Ln/Exp reciprocal.
    from concourse.hw_specs import get_activation_tables as _gat

    def _patched_tables(arch):
        tabs = _gat(arch)
        key = "natural_log_exp_and_others"
        comb = tabs[key]
        return {n: (s if n == key else (s - comb)) for n, s in tabs.items()}

    _orig_gat = bacc.get_activation_tables
    bacc.get_activation_tables = _patched_tables

    nc = bacc.Bacc("TRN2", target_bir_lowering=False, debug=False,
                   num_devices=NCORES)
    # x stored j-major, partition-row major: xTt[j*128+p, d*512+c] holds
    # x^T element (d*128+p, j*512+c) -> each j-chunk is one plain 2D DMA
    xTt = nc.dram_tensor("xTt", [8 * 128, 16 * 512], BF16,
                         kind="ExternalInput").ap()
    wqkvT = nc.dram_tensor("wqkvT", [D, 384], BF16, kind="ExternalInput").ap()
    woT = nc.dram_tensor("woT", [256, D], BF16, kind="ExternalInput").ap()
    c4 = nc.dram_tensor("c4", [128, S], BF16, kind="ExternalInput").ap()
    s4 = nc.dram_tensor("s4", [128, S], BF16, kind="ExternalInput").ap()
    maskT = nc.dram_tensor("maskT", [128, 256], BF16,
                           kind="ExternalInput").ap()
    o = nc.dram_tensor("o", [T, D], BF16, kind="ExternalOutput").ap()

    with tile.TileContext(nc) as tc:
        with tc.tile_pool(name="res", bufs=1) as res, \
             tc.tile_pool(name="xtp", bufs=2) as xtp, \
             tc.tile_pool(name="ropet", bufs=2) as rp, \
             tc.tile_pool(name="probs", bufs=4) as probsp, \
             tc.tile_pool(name="attnp", bufs=2) as attnp, \
             tc.tile_pool(name="normp", bufs=2) as normp, \
             tc.tile_pool(name="outp", bufs=2) as outp, \
             tc.tile_pool(name="mmps", bufs=2, space="PSUM") as mmps, \
             tc.tile_pool(name="sps", bufs=2, space="PSUM") as sps, \
             tc.tile_pool(name="pvps", bufs=1, space="PSUM") as pvps:

            ident64 = res.tile([64, 64], BF16)
            make_identity(nc, ident64[:])
            c4_sb = res.tile([128, S], BF16)
            s4_sb = res.tile([128, S], BF16)
            maskT_sb = res.tile([128, 256], BF16)
            maskT3 = maskT_sb.rearrange("p (h c) -> p h c", c=128)

            # per-512-token-chunk resident tiles keep dependency tracking
            # exact: a pair's scores must never falsely wait on a LATER
            # chunk's rope/V-transpose writes to a shared tile
            QRI_As = [res.tile([128, 512], BF16, name=f"qriA{j}")
                      for j in range(8)]       # [h0r h0i h1r h1i] x tok
            QRI_Bs = [res.tile([128, 512], BF16, name=f"qriB{j}")
                      for j in range(8)]       # [h2r h2i h3r h3i]
            KRI2s = [res.tile([128, 512], BF16, name=f"kri{j}")
                     for j in range(8)]        # [Kr Ki Kr Ki]
            Vts = [res.tile([128, 4 * 65], BF16, name=f"vt{j}")
                   for j in range(8)]          # 4 kt-slots of 65 cols
            Vt3s = [v.rearrange("p (k c) -> p k c", c=65) for v in Vts]
            wqkv_r = res.tile([128, 16 * 384], BF16)
            woT_r = res.tile([128, 2 * D], BF16)
            ones32 = res.tile([128, 32], BF16)
            nc.gpsimd.memset(ones32[:], 1.0)
            for j in range(8):
                nc.vector.tensor_copy(Vt3s[j][:, :, 64], ones32[:, 0:4])
            ones64 = res.tile([1, 64], BF16)
            nc.gpsimd.memset(ones64[:], 1.0)
            warmM = res.tile([128, 512], BF16)
            nc.gpsimd.memset(warmM[:], 0.0)

            # PE warm-up: real matmuls are DMA-paced until ~16us, so a
            # ~4us warm burst lifts the HAM clock gate to 2.4GHz right
            # as the first projection matmuls issue.
            for _ in range(10):
                wps = sps.tile([128, 1024], F32, name="Sg")
                nc.tensor.matmul(wps[:, 0:512], warmM[:, 0:128], warmM[:],
                                 start=True, stop=True)
            # ---- DMA plan: Sync queue carries x tiles + o writes + norm
            # reshapes; Scalar queue carries only the weight preload (so
            # phase-2 exps are never stuck behind DMA blocks).
            xt_tiles = {}

            def issue_xt(j):
                xall = xtp.tile([128, 16 * 512], BF16, name="xall")
                if j < 2:
                    # per-tile loads so proj(j)'s d-loop starts asap
                    for d in range(16):
                        nc.sync.dma_start(
                            xall[:, d * 512:(d + 1) * 512],
                            xTt[j * 128:(j + 1) * 128,
                                d * 512:(d + 1) * 512])
                else:
                    # prefetched a pair ahead; 4 x 512KB so norm-chain
                    # DMAs queued behind never wait more than ~1.4us
                    for q in range(4):
                        nc.sync.dma_start(
                            xall[:, q * 2048:(q + 1) * 2048],
                            xTt[j * 128:(j + 1) * 128,
                                q * 2048:(q + 1) * 2048])
                xt_tiles[j] = xall
                if j == 0:
                    for d in range(16):
                        nc.scalar.dma_start(
                            wqkv_r[:, d * 384:(d + 1) * 384],
                            wqkvT[d * 128:(d + 1) * 128, :])
                    nc.scalar.dma_start(c4_sb[:], c4[:])
                    nc.scalar.dma_start(s4_sb[:], s4[:])
                elif j == 1:
                    nc.sync.dma_start(maskT_sb[:], maskT[:])
                    for t in range(2):
                        nc.sync.dma_start(woT_r[:, t * D:(t + 1) * D],
                                          woT[t * 128:(t + 1) * 128, :])

            issue_xt(0)

            # ---------------- projection + rope for one 512-token chunk
            def proj(j):
                xall = xt_tiles.pop(j)
                kb = rp.tile([64, 512], BF16, name="kb")
                vsb = rp.tile([64, 512], BF16, name="vsb")
                qrb = rp.tile([128, 512], BF16, name="qrb")
                qib = rp.tile([128, 512], BF16, name="qib")
                for ch in range(3):
                    ps = mmps.tile([128, 512], F32, name="mm")
                    for d in range(16):
                        nc.tensor.matmul(
                            ps[:],
                            wqkv_r[:, d * 384 + ch * 128:
                                   d * 384 + (ch + 1) * 128],
                            xall[:, d * 512:(d + 1) * 512],
                            start=(d == 0), stop=(d == 15))
                    # stage before the next tile() call recycles the slot
                    if ch == 0:
                        nc.scalar.copy(qrb[:], ps[:])
                    elif ch == 1:
                        nc.scalar.copy(qib[:], ps[:])
                    else:
                        nc.scalar.copy(kb[:], ps[0:64, :])
                        nc.scalar.copy(vsb[:], ps[64:128, :])
                    drain_wo(1)
                # V transpose tiles (PE, psum slot shared with proj/wo)
                vtp = mmps.tile([128, 256], BF16, name="vtp", tag="mm")
                for i in range(4):
                    nc.tensor.transpose(
                        vtp[:, i * 64:(i + 1) * 64],
                        vsb[:, i * 128:(i + 1) * 128], ident64[:])
                vtp3 = vtp.rearrange("p (k c) -> p k c", c=64)
                nc.vector.tensor_copy(Vt3s[j][:, :, 0:64], vtp3[:])
                drain_wo(1)

                def rope():
                    tb = 0
                    bc = (j % 4) * 512
                    cs = c4_sb[:, bc:bc + 512]
                    sn = s4_sb[:, bc:bc + 512]
                    cs32 = c4_sb[0:32, bc:bc + 512]
                    sn32 = s4_sb[0:32, bc:bc + 512]
                    u1 = rp.tile([32, 512], BF16, name="u1", bufs=1)
                    u2 = rp.tile([32, 512], BF16, name="u2", bufs=1)
                    u3 = rp.tile([32, 512], BF16, name="u3", bufs=1)
                    u4 = rp.tile([32, 512], BF16, name="u4", bufs=1)
                    cs32b = c4_sb[32:64, bc:bc + 512]
                    sn32b = s4_sb[32:64, bc:bc + 512]
                    nc.vector.tensor_mul(u1[:], kb[0:32, :], cs32)
                    nc.vector.tensor_mul(u2[:], kb[32:64, :], sn32b)
                    nc.vector.tensor_mul(u3[:], kb[0:32, :], sn32)
                    nc.vector.tensor_mul(u4[:], kb[32:64, :], cs32b)
                    for g in (0, 64):
                        nc.vector.tensor_sub(
                            KRI2s[j][g:g + 32, tb:tb + 512], u1[:], u2[:])
                    for g in (32, 96):
                        nc.vector.tensor_add(
                            KRI2s[j][g:g + 32, tb:tb + 512], u3[:], u4[:])
                    t1 = rp.tile([128, 512], BF16, name="t1", bufs=1)
                    t2 = rp.tile([128, 512], BF16, name="t2", bufs=1)
                    t3 = rp.tile([128, 512], BF16, name="t3", bufs=1)
                    t4 = rp.tile([128, 512], BF16, name="t4", bufs=1)
                    nc.vector.tensor_mul(t1[:], qrb[:], cs)
                    nc.vector.tensor_mul(t3[:], qrb[:], sn)
                    nc.vector.tensor_mul(t2[:], qib[:], sn)
                    nc.vector.tensor_mul(t4[:], qib[:], cs)
                    for hh in range(4):
                        dst = QRI_As[j] if hh < 2 else QRI_Bs[j]
                        base = (hh % 2) * 64
                        nc.vector.tensor_sub(
                            dst[base:base + 32, tb:tb + 512],
                            t1[32 * hh:32 * hh + 32, :],
                            t2[32 * hh:32 * hh + 32, :])
                        nc.vector.tensor_add(
                            dst[base + 32:base + 64, tb:tb + 512],
                            t3[32 * hh:32 * hh + 32, :],
                            t4[32 * hh:32 * hh + 32, :])
                return rope

            # ---------------- attention machinery
            pending = deque()
            norm_q = deque()
            nprog = {"enq": 0, "pi_done": 0}
            NPAIRS = 8

            def enqueue_wo(attn01, attn23, qb):
                state = {}
                pid = nprog["enq"]
                nprog["enq"] += 1
                for qs in range(4):
                    for do in range(4):
                        pending.append(
                            (pid, attn01, attn23, qb, qs, do, state))

            def drain_wo(n, reserve=0):
                for _ in range(n):
                    if len(pending) <= reserve:
                        return
                    if pending[0][0] >= nprog["pi_done"] // 2:
                        return   # this pair's attn not normed yet
                    pid, attn01, attn23, qb, qs, do, state = \
                        pending.popleft()
                    last = pid == NPAIRS - 1
                    if do == 0:
                        state[qs] = outp.tile([128, D], BF16, name="osb")
                    osb = state[qs]
                    Ops = mmps.tile([128, 512], F32, name="mm")
                    nc.tensor.matmul(
                        Ops[:], attn01[:, qs * 128:(qs + 1) * 128],
                        woT_r[:, do * 512:(do + 1) * 512],
                        start=True, stop=False)
                    nc.tensor.matmul(
                        Ops[:], attn23[:, qs * 128:(qs + 1) * 128],
                        woT_r[:, D + do * 512:D + (do + 1) * 512],
                        start=False, stop=True)
                    if last and do % 2 == 1:
                        nc.scalar.copy(
                            osb[:, do * 512:(do + 1) * 512], Ops[:])
                    else:
                        nc.vector.tensor_copy(
                            osb[:, do * 512:(do + 1) * 512], Ops[:])
                    if do == 3:
                        qq = qb + qs * 128
                        eng = nc.scalar if (last and qs % 2) else nc.sync
                        eng.dma_start(o[qq:qq + 128, :], osb[:])

            def emit_pv(PVs, b, pkt, ppg, pcsl, stop):
                vt = Vt3s[b * 4 + pkt // 4][:, pkt % 4, :]
                st = (pkt == 0)
                for hh in range(2):
                    hs = hh * 512
                    nc.tensor.matmul(
                        PVs[:, hs + pcsl.start:hs + pcsl.stop], vt,
                        ppg[:, hs + pcsl.start:hs + pcsl.stop],
                        start=st, stop=stop)

            def make_norm(pvc, pi, attn, pe_bcst=False):
                st = {}
                cb = 1024 * pi

                def mul_stage(bcst):
                    for hh in range(2):
                        nc.vector.tensor_mul(
                            attn[64 * hh:64 * hh + 64, :],
                            pvc[0:64, cb + 512 * hh:cb + 512 * hh + 512],
                            bcst[:, 512 * hh:512 * hh + 512])
                    nprog["pi_done"] += 1

                if pe_bcst:
                    # tail chains: every DMA hop costs ~2us of completion
                    # receipt latency, so run Ln/Exp directly on the
                    # [1,1024] denominator row and broadcast on the
                    # (idle) PE via a K=1 bf16 outer product.
                    def t1():
                        st["l1"] = normp.tile([1, 1024], F32, name="l1")
                        nc.scalar.activation(st["l1"][:],
                                             pvc[64:65, cb:cb + 1024],
                                             AF.Ln)

                    def t2():
                        st["r1"] = normp.tile([1, 1024], BF16, name="r1")
                        nc.scalar.activation(st["r1"][:], st["l1"][:],
                                             AF.Exp, scale=-1.0)

                    def t3():
                        bcst = sps.tile([64, 1024], F32, name="Sg",
                                        tag="Sg")
                        for hh in range(2):
                            nc.tensor.matmul(
                                bcst[:, 512 * hh:512 * hh + 512],
                                ones64[:],
                                st["r1"][:, 512 * hh:512 * hh + 512],
                                start=True, stop=True)
                        mul_stage(bcst)
                    return [t1, t2, t3]

                def s1():
                    st["d8"] = normp.tile([8, 128], F32, name="d8")
                    nc.sync.dma_start(st["d8"][:],
                                      pvc[64:65, cb:cb + 1024])

                def s2():
                    l8 = normp.tile([8, 128], F32, name="l8")
                    nc.scalar.activation(l8[:], st["d8"][:], AF.Ln)
                    st["r8"] = normp.tile([8, 128], BF16, name="r8")
                    nc.scalar.activation(st["r8"][:], l8[:],
                                         AF.Exp, scale=-1.0)

                def s3():
                    rec1 = normp.tile([1, 1024], BF16, name="rec1")
                    nc.sync.dma_start(rec1[:], st["r8"][:])
                    bcst = normp.tile([64, 1024], BF16, name="bc")
                    nc.gpsimd.partition_broadcast(bcst[:], rec1[:])
                    mul_stage(bcst)
                return [s1, s2, s3]

            def attention(b, jp, rope_mid=None):
                qb = b * S + jp * 512
                nkt = 4 * jp + 4
                # during the final pair, hold back a few normed wo steps so
                # the tail's norm-latency window has PE work queued
                rsv = 6 if (b, jp) == (1, 3) else 0
                attn01 = attnp.tile([128, 512], BF16, name="at01")
                attn23 = attnp.tile([128, 512], BF16, name="at23")
                pvc = normp.tile([65, 2048], F32, name="pvc")
                QRIs = (QRI_As[b * 4 + jp], QRI_Bs[b * 4 + jp])
                attns = (attn01, attn23)
                pgqs = (deque(), deque())

                def emit_score(pi, kt):
                    kri = KRI2s[b * 4 + kt // 4]
                    ko = (kt % 4) * 128
                    r = kt - (nkt - 4)
                    cs0 = max(0, 128 * r)   # live q-col start
                    Sg = sps.tile([128, 1024], F32, name="Sg")
                    pg = probsp.tile([128, 1024], BF16, name="pg")
                    for hh in range(2):
                        hs = hh * 512
                        nc.tensor.matmul(
                            Sg[:, hs + cs0:hs + 512],
                            kri[64 * hh:64 * hh + 64, ko:ko + 128],
                            QRIs[pi][64 * hh:64 * hh + 64, cs0:512],
                            start=True, stop=True,
                            tile_position=(64 * hh, 0))
                    sgv = Sg.rearrange(
                        "p (h c) -> p h c", h=2)[:, :, cs0:512]
                    pgv = pg.rearrange(
                        "p (h c) -> p h c", h=2)[:, :, cs0:512]
                    nc.scalar.activation(pgv, sgv, AF.Exp, scale=0.125)
                    if r >= 0:
                        pgt = pg.rearrange(
                            "p (h c) -> p h c", h=2)[:, :, cs0:cs0 + 128]
                        nc.vector.tensor_mul(pgt, pgt, maskT3[:])
                    pgqs[pi].append((kt, pg, slice(cs0, 512)))

                def finish_pi(pi, PVs):
                    pgq = pgqs[pi]
                    while pgq:
                        item = pgq.popleft()
                        emit_pv(PVs, b, *item, stop=not pgq)
                        drain_wo(1, rsv)
                    if norm_q:
                        norm_q.popleft()()
                    # free the PV banks asap; norm (per pi) is deferred
                    # into the following kt loops
                    nc.vector.tensor_copy(
                        pvc[:, 1024 * pi:1024 * pi + 1024], PVs[:])
                    norm_q.extend(make_norm(pvc, pi, attns[pi],
                                            pe_bcst=rsv > 0))

                # ---- pi0 ----
                drain_wo(2, rsv)
                PVs0 = pvps.tile([65, 1024], F32, name="PV")
                for kt in range(nkt):
                    # drains before the score so their DVE casts never sit
                    # ahead of this kt's mask mul (which gates its PV)
                    drain_wo(2 if len(pending) > 8 else 1, rsv)
                    emit_score(0, kt)
                    if len(pgqs[0]) > 2:
                        emit_pv(PVs0, b, *pgqs[0].popleft(), stop=False)
                    if kt in (1, 2, 3) and norm_q:
                        norm_q.popleft()()
                # hoist pi1's first scores to cover pi0's exp tail
                emit_score(1, 0)
                drain_wo(1, rsv)
                emit_score(1, 1)
                finish_pi(0, PVs0)
                if rope_mid is not None:
                    rope_mid()
                # ---- pi1 ----
                PVs1 = pvps.tile([65, 1024], F32, name="PV")
                for kt in range(2, nkt):
                    drain_wo(2 if len(pending) > 8 else 1, rsv)
                    emit_score(1, kt)
                    if len(pgqs[1]) > 2:
                        emit_pv(PVs1, b, *pgqs[1].popleft(), stop=False)
                    if kt in (2, 3, 4) and norm_q:
                        norm_q.popleft()()
                finish_pi(1, PVs1)
                drain_wo(2, rsv)
                enqueue_wo(attn01, attn23, qb)

            # ---------------- fused emission stream
            stream = [("p", 0), ("p", 1), ("a", 0, 0), ("p", 2),
                      ("a", 0, 1), ("p", 3), ("a", 0, 2), ("p", 4),
                      ("a", 0, 3), ("p", 5), ("a", 1, 0), ("p", 6),
                      ("a", 1, 1), ("p", 7), ("a", 1, 2), ("a", 1, 3)]
            next_xt = 2
            rope_pend = None
            for step in stream:
                if step[0] == "p":
                    j = step[1]
                    rope_fn = proj(j)
                    if j < 2:
                        # needed by the very next pair; emit inline
                        rope_fn()
                    else:
                        rope_pend = rope_fn
                    if j == 0:
                        issue_xt(1)
                else:
                    if next_xt < 8:
                        issue_xt(next_xt)
                        next_xt += 1
                    # rope for chunk j is emitted mid-pair (between pi0
                    # and pi1) so it never sits ahead of the pair's norm
                    # muls or next pair's masks on the DVE queue
                    attention(step[1], step[2], rope_mid=rope_pend)
                    rope_pend = None
            # emit any ungated wo steps (the last pair's reserve) BEFORE
            # the final norm chain so the PE has work during its latency;
            # high_priority makes the list scheduler place them as early
            # as their deps allow instead of after the norm chain
            with tc.high_priority():
                drain_wo(1 << 30)
            while norm_q:
                norm_q.popleft()()
            drain_wo(1 << 30)

    nc.compile()
    bacc.get_activation_tables = _orig_gat
    return nc


def _prep_inputs(x, freqs_cos, freqs_sin, wq, wk, wv, wo):
    from ml_dtypes import bfloat16
    xf = np.asarray(x, np.float32).reshape(T, D)
    xTf = np.ascontiguousarray(xf.T).astype(bfloat16)      # [D, T]
    # j-major p-row-major layout: xTt[j, p, d, c] = xT[d*128+p, j*512+c]
    xTt = np.ascontiguousarray(
        xTf.reshape(16, 128, 8, 512).transpose(2, 1, 0, 3)
    ).reshape(8 * 128, 16 * 512)
    wq = np.asarray(wq, np.float32)
    wk = np.asarray(wk, np.float32)
    wv = np.asarray(wv, np.float32)
    wo = np.asarray(wo, np.float32)
    fc = np.asarray(freqs_cos, np.float32)
    fs = np.asarray(freqs_sin, np.float32)

    c4 = np.ascontiguousarray(np.tile(fc.T, (4, 1))).astype(bfloat16)
    s4 = np.ascontiguousarray(np.tile(fs.T, (4, 1))).astype(bfloat16)
    kt = np.arange(128)[:, None]
    qt = np.arange(128)[None, :]
    tri = (kt <= qt).astype(np.float32)
    maskT = np.ascontiguousarray(np.tile(tri, (1, 2))).astype(bfloat16)
    ev = np.arange(0, 64, 2)
    od = np.arange(1, 64, 2)

    in_maps = []
    for c in range(NCORES):
        qreal = np.concatenate([(4 * c + h) * 64 + ev for h in range(4)])
        qimag = np.concatenate([(4 * c + h) * 64 + od for h in range(4)])
        Wc = np.concatenate([wq[qreal], wq[qimag], wk[c * 64 + ev],
                             wk[c * 64 + od], wv[c * 64:(c + 1) * 64]], axis=0)
        in_maps.append({
            "xTt": xTt,
            "wqkvT": np.ascontiguousarray(Wc.T).astype(bfloat16),
            "woT": np.ascontiguousarray(
                wo[:, c * 256:(c + 1) * 256].T).astype(bfloat16),
            "c4": c4, "s4": s4, "maskT": maskT,
        })
    return in_maps


def _run(in_maps, trace=False, **kw):
    from concourse import bass_utils
    if "nc" not in _cache:
        _cache["nc"] = _build()
    return bass_utils.run_bass_kernel_spmd(
        _cache["nc"], in_maps, core_ids=list(range(NCORES)), trace=trace, **kw)


def kernel(x, freqs_cos, freqs_sin, wq, wk, wv, wo):
    in_maps = _prep_inputs(x, freqs_cos, freqs_sin, wq, wk, wv, wo)
    res = _run(in_maps)
    out = np.zeros((T, D), np.float64)
    for c in range(NCORES):
        out += np.asarray(res.results[c]["o"], np.float32)
    return out.astype(np.float32).reshape(B, S, D)
